# revision 48
# baseline (speedup 1.0000x reference)
"""BOW classifier kernel for 8 Trainium2 NeuronCores.

Vocab-sharded counts-matmul formulation.  The masked mean-pool
  pooled[b] = (1/len[b]) * sum_{s<len[b]} emb[text[s,b]]
is a sparse matmul  pooled = counts @ emb  with counts[b,v] the number of
times token v appears in the first len[b] positions of column b (the
1/len is folded into counts on the host).  Each core owns a 6272-row
slice of the (padded, bf16) embedding table and the matching slice of
counts^T, computes its partial pooled on the tensor engine (bf16 x bf16
-> fp32 PSUM), and a bf16 ReduceScatter sums the partials and hands core
i batch rows [128*i, 128*(i+1)).  The MLP tail runs per-core on its 128
batch rows: pooled^T lands via XBAR DMA-transpose straight out of the
collective buffer, fc1 computes h^T = relu(W1^T pooled^T + b1) so fc2
(out = h @ W2 + b2) needs no transposes at all; bf16 inputs, fp32 PSUM.

Schedule notes: dummy matmuls on memset tiles ramp the PE pstate during
the initial DMA fill (the real accumulation opens with start=True, so
the junk is discarded); count/embedding DMAs interleave across the two
HWDGE queues (sync + scalar; counts two 128-row chunks per instruction)
while gpsimd carries only the small transfers, keeping every issue path
ahead of the PE's ~1.0 us/chunk consume rate.  The ReduceScatter
triggers as soon as the accumulator drains land; its start is pinned by
NRT's fixed first-collective barrier (~70 us), which the matmul phase
hides.
"""

import sys

import numpy as np

for _p in ("/opt/trn_rl_repo",):
    if _p not in sys.path:
        sys.path.insert(0, _p)

V, E, H, O = 50000, 300, 512, 2
S, B = 512, 1024
NCORES = 8
VQ = 4              # vocab quarters (core i = batch half i//4, quarter i%4)
BH = 2              # batch halves
VSH = 12544         # padded vocab rows per core (98 * 128)
VP = VQ * VSH       # 50176 padded vocab rows total
KC = VSH // 128     # 98 contraction chunks per core
BSH = B // BH       # 512 batch columns per core
BG = BSH // 128     # 4 local batch groups of 128
BS = B // NCORES    # 128 batch rows per core after reduce-scatter
NWARM = 8           # dummy matmuls to start the PE pstate ramp
OV = 256 - 172      # zeroed overlap rows in the third fc1 weight chunk


def _build_nc(repeat=None, cnt_fp8=True):
    import os
    from contextlib import ExitStack

    if repeat is None:
        repeat = int(os.environ.get("KERNEL_REPEAT", "1"))

    import concourse.tile as tile
    from concourse import bacc, bass, mybir

    bf16, f32 = mybir.dt.bfloat16, mybir.dt.float32
    cdt = mybir.dt.float8e4 if cnt_fp8 else bf16

    nc = bacc.Bacc(None, target_bir_lowering=False, num_devices=NCORES)
    cnt_d = nc.declare_dram_parameter("cnt", [VSH, BSH], cdt, isOutput=False)
    emb_d = nc.declare_dram_parameter("emb", [VSH, E], bf16, isOutput=False)
    il_d = nc.declare_dram_parameter("ivl", [128, BG], f32, isOutput=False)
    # rows 0:384 = three 128-row lhsT chunks of W1 (the third covers
    # pooled^T rows 172:300 with the 84 overlap rows zeroed), row 384 = b1
    w1b_d = nc.declare_dram_parameter("w1b", [385, H], bf16, isOutput=False)
    w2b_d = nc.declare_dram_parameter("w2b", [H + 1, O], bf16, isOutput=False)
    out_d = nc.declare_dram_parameter("out", [BS, O], f32, isOutput=True)

    with tile.TileContext(nc) as tc, ExitStack() as ctx:
        sb = ctx.enter_context(tc.tile_pool(name="sb", bufs=1))
        dram = ctx.enter_context(tc.tile_pool(name="dram", bufs=1, space="DRAM"))

        # tiny warm-up collective, triggered as early as possible: every
        # core reaches NRT's first-collective rendezvous within ~10 us of
        # its NEFF start, so cross-core launch skew and the CC stream
        # setup are absorbed while the matmul phase runs (the real
        # ReduceScatter then starts trigger-bound, not barrier-bound).
        # The input is an uninitialized scratch tile (nobody reads the
        # result), so the trigger carries no data dependency at all.
        warm_in = dram.tile([4, 64], bf16)
        warm_out = dram.tile([1, 64], bf16)
        nc.gpsimd.collective_compute(
            "ReduceScatter",
            mybir.AluOpType.add,
            replica_groups=[[0, 1, 2, 3], [4, 5, 6, 7]],
            ins=[warm_in.opt()],
            outs=[warm_out.opt()],
        )

        # counts (two 128-row chunks per instruction) and embeddings,
        # interleaved in chunk order across the two HWDGE queues
        cnt_t, emb_t = [], []
        for j in range((KC + 1) // 2):
            r1 = min((j + 1) * 256, VSH)
            t2 = (r1 - j * 256) // 128
            ct = sb.tile([128, t2 * BSH], cdt, tag=f"cnt{j}", name=f"cnt{j}")
            eng_c = nc.sync if j % 2 == 0 else nc.scalar
            eng_e = nc.scalar if j % 2 == 0 else nc.sync
            eng_c.dma_start(
                out=ct[:].rearrange("p (t c) -> p t c", t=t2),
                in_=cnt_d[j * 256:r1, :].rearrange("(t p) c -> p t c", t=t2),
            )
            cnt_t.append(ct)
            et = sb.tile([128, t2 * E], bf16, tag=f"emb{j}", name=f"emb{j}")
            eng_e.dma_start(
                out=et[:].rearrange("p (t c) -> p t c", t=t2),
                in_=emb_d[j * 256:r1, :].rearrange("(t p) c -> p t c", t=t2),
            )
            emb_t.append(et)

        w1_t = []
        for c in range(3):
            t = sb.tile([128, H], bf16, tag=f"w1_{c}", name=f"w1_{c}")
            nc.gpsimd.dma_start(out=t[:], in_=w1b_d[c * 128:(c + 1) * 128, :])
            w1_t.append(t)
        b1_t = sb.tile([1, H], bf16, tag="b1")
        nc.gpsimd.dma_start(out=b1_t[:], in_=w1b_d[384:385, :])
        w2_t = []
        for c in range(4):
            t = sb.tile([128, O], bf16, tag=f"w2_{c}", name=f"w2_{c}")
            nc.gpsimd.dma_start(out=t[:], in_=w2b_d[c * 128:(c + 1) * 128, :])
            w2_t.append(t)
        b2_t = sb.tile([1, O], bf16, tag="b2")
        nc.gpsimd.dma_start(out=b2_t[:], in_=w2b_d[H:H + 1, :])
        ivl = sb.tile([128, BG], f32, tag="ivl")
        nc.gpsimd.dma_start(out=ivl[:], in_=il_d[:])

        # PE pstate warm-up on memset tiles (no DMA dependency); the real
        # accumulation below opens with start=True, discarding this junk
        wa = sb.tile([128, 128], bf16, tag="wa")
        nc.vector.memset(wa[:], 0.0)
        wb = sb.tile([128, E], bf16, tag="wb")
        nc.vector.memset(wb[:], 0.0)
        ones1 = sb.tile([1, 128], bf16, tag="ones1")
        nc.vector.memset(ones1[:], 1.0)

        pooled_all = sb.tile([128, BG * E], bf16, tag="pooled_all")
        with tc.tile_pool(name="psA", bufs=1, space="PSUM") as psA:
            acc = [
                psA.tile([128, 512], f32, tag=f"acc{g}", name=f"acc{g}")
                for g in range(BG)
            ]
            for w in range(NWARM):
                nc.tensor.matmul(out=acc[0][:, 0:E], lhsT=wa[:], rhs=wb[:],
                                 start=True, stop=True)
            # group order: bounce piece A's groups (4-7) stop and drain
            # first, piece B's (0-2) next, and g3 last so the trigger's
            # final wait covers only the tiny last piece
            GORD = (2, 3, 0, 1)
            for rep in range(repeat):
                for k in range(KC):
                    ct = cnt_t[k // 2]
                    t = k % 2
                    et = emb_t[k // 2]
                    for g in GORD:
                        nc.tensor.matmul(
                            out=acc[g][:, 0:E],
                            lhsT=ct[:, t * BSH + g * 128:t * BSH + (g + 1) * 128],
                            rhs=et[:, t * E:(t + 1) * E],
                            start=(k == 0),
                            stop=(k == KC - 1),
                        )
            # drain the accumulators, folding in the 1/len scale (vector +
            # scalar in parallel; pipelines behind the last matmuls)
            for g in GORD:
                dst = pooled_all[:, g * E:(g + 1) * E]
                if g % 2 == 0:
                    nc.vector.tensor_scalar(
                        out=dst, in0=acc[g][:, 0:E],
                        scalar1=ivl[:, g:g + 1], scalar2=None,
                        op0=mybir.AluOpType.mult,
                    )
                else:
                    nc.scalar.activation(
                        out=dst, in_=acc[g][:, 0:E],
                        func=mybir.ActivationFunctionType.Copy,
                        scale=ivl[:, g:g + 1],
                    )

        # cross-core sum + scatter: core i keeps batch rows [128i, 128i+128).
        # Bounce in three pieces across three queues, staged behind the
        # drains, so the collective trigger's last completion wait is a
        # single-group DMA.
        part_d = dram.tile([BSH, E], bf16)
        rs_d = dram.tile([BS, E], bf16)
        nc.gpsimd.dma_start(
            out=part_d[2 * 128:, :].rearrange("(g p) e -> p g e", g=2),
            in_=pooled_all[:, 2 * E:].rearrange("p (g e) -> p g e", g=2),
        )
        nc.sync.dma_start(
            out=part_d[0:128, :],
            in_=pooled_all[:, 0:E],
        )
        nc.scalar.dma_start(
            out=part_d[128:256, :],
            in_=pooled_all[:, E:2 * E],
        )
        nc.gpsimd.collective_compute(
            "ReduceScatter",
            mybir.AluOpType.add,
            replica_groups=[[0, 1, 2, 3], [4, 5, 6, 7]],
            ins=[part_d.opt()],
            outs=[rs_d.opt()],
        )

        with tc.tile_pool(name="ps", bufs=1, space="PSUM") as ps:
            # pooled^T as three 128-wide XBAR DMA-transposes straight from
            # rs_d; the third chunk (cols 172:300) overlaps the second, and
            # the matching fc1 weight rows are zeroed host-side
            lhs = []
            for c, c0 in enumerate([0, 128, E - 128]):
                lt = sb.tile([128, 128], bf16, tag=f"lhs{c}", name=f"lhs{c}")
                eng = nc.scalar if c == 1 else nc.sync
                eng.dma_start_transpose(lt[:], rs_d[:, c0:c0 + 128])
                lhs.append(lt)

            # fc1 flipped: hT[hc] = W1[:, hc]^T @ pooled^T, bias via ones row
            hT_ps = [
                ps.tile([128, 128], f32, tag=f"hT{hc}", name=f"hT{hc}")
                for hc in range(4)
            ]
            for hc in range(4):
                for c in range(3):
                    nc.tensor.matmul(
                        out=hT_ps[hc][:],
                        lhsT=w1_t[c][:, hc * 128:(hc + 1) * 128],
                        rhs=lhs[c][:],
                        start=(c == 0), stop=False,
                    )
                nc.tensor.matmul(
                    out=hT_ps[hc][:],
                    lhsT=b1_t[:, hc * 128:(hc + 1) * 128],
                    rhs=ones1[:],
                    start=False, stop=True,
                )
            hT = sb.tile([128, 4 * 128], bf16, tag="hT")
            for hc in range(4):
                dst = hT[:, hc * 128:(hc + 1) * 128]
                if hc % 2 == 0:
                    nc.scalar.activation(
                        out=dst, in_=hT_ps[hc][:],
                        func=mybir.ActivationFunctionType.Relu)
                else:
                    nc.vector.tensor_scalar(
                        out=dst, in0=hT_ps[hc][:], scalar1=0.0, scalar2=None,
                        op0=mybir.AluOpType.max)

            # fc2: out = h @ W2 + b2 (hT is already the needed lhsT)
            op_ = ps.tile([128, O], f32, tag="op", space="PSUM")
            for c in range(4):
                nc.tensor.matmul(
                    out=op_[:], lhsT=hT[:, c * 128:(c + 1) * 128],
                    rhs=w2_t[c][:], start=(c == 0), stop=False)
            nc.tensor.matmul(out=op_[:], lhsT=ones1[:], rhs=b2_t[:],
                             start=False, stop=True)
            out_sb = sb.tile([BS, O], f32, tag="osb")
            nc.vector.tensor_copy(out=out_sb[:], in_=op_[:])
            # scalar HWDGE queue: still warm from the XBAR transpose a few
            # us earlier, so the completion semaphore lands in ~2 us instead
            # of the ~6 us an idle queue takes
            nc.scalar.dma_start(out=out_d[:], in_=out_sb[:])

    nc.finalize()
    return nc


def _prep_in_maps(text, lengths, emb_table, W1, b1, W2, b2):
    import ml_dtypes

    bf16 = ml_dtypes.bfloat16
    text = np.asarray(text, dtype=np.int64)         # [S, B]
    lengths = np.asarray(lengths, dtype=np.int64)   # [B]

    # counts^T [VP, B]: row v = per-batch count of token v among the
    # first len[b] positions (vocab-major for sharding); the 1/len scale
    # is applied on-device at accumulator-drain time
    mask = np.arange(S)[:, None] < lengths[None, :]
    flat = (text * B + np.arange(B)[None, :])[mask]
    cntT = np.bincount(flat, minlength=VP * B).reshape(VP, B)
    cnt_fp8 = cntT.max() <= 15  # integers <= 15 are exact in fp8e4m3
    cdt = ml_dtypes.float8_e4m3fn if cnt_fp8 else bf16
    cntT16 = cntT.astype(cdt)
    inv_len = (1.0 / lengths.astype(np.float32)).astype(np.float32)

    embp = np.zeros((VP, E), np.float32)
    embp[:V] = np.asarray(emb_table, np.float32)
    emb16 = embp.astype(bf16)

    # fc1 lhsT chunks: [0:128), [128:256), [172:300) with the 84 rows that
    # overlap chunk 1 zeroed, then the b1 row
    W1f = np.asarray(W1, np.float32)
    c2 = np.zeros((128, H), np.float32)
    c2[OV:] = W1f[256:E]
    w1b = np.vstack([W1f[0:128], W1f[128:256], c2,
                     np.asarray(b1, np.float32)[None, :]]).astype(bf16)
    w2b = np.vstack([np.asarray(W2, np.float32),
                     np.asarray(b2, np.float32)[None, :]]).astype(bf16)

    in_maps = []
    for i in range(NCORES):
        hb, q = i // VQ, i % VQ
        in_maps.append({
            "cnt": np.ascontiguousarray(
                cntT16[q * VSH:(q + 1) * VSH, hb * BSH:(hb + 1) * BSH]),
            "emb": np.ascontiguousarray(emb16[q * VSH:(q + 1) * VSH]),
            "ivl": np.ascontiguousarray(
                inv_len[hb * BSH:(hb + 1) * BSH].reshape(BG, 128).T),
            "w1b": w1b,
            "w2b": w2b,
        })
    return in_maps, cnt_fp8


def _run(inputs, trace=False):
    from concourse.bass_utils import run_bass_kernel_spmd

    in_maps, cnt_fp8 = _prep_in_maps(**inputs)
    nc = _build_nc(cnt_fp8=cnt_fp8)
    res = run_bass_kernel_spmd(nc, in_maps, list(range(NCORES)), trace=trace)
    out = np.concatenate([res.results[i]["out"] for i in range(NCORES)], axis=0)
    return out.astype(np.float32), res


def kernel(**inputs):
    out, _ = _run(inputs, trace=False)
    return out


# revision 49
# speedup vs baseline: 1.4236x; 1.4236x over previous
"""BOW classifier kernel for 8 Trainium2 NeuronCores.

Vocab-sharded counts-matmul formulation.  The masked mean-pool
  pooled[b] = (1/len[b]) * sum_{s<len[b]} emb[text[s,b]]
is a sparse matmul  pooled = counts @ emb  with counts[b,v] the number of
times token v appears in the first len[b] positions of column b (the
1/len is folded into counts on the host).  Each core owns a 6272-row
slice of the (padded, bf16) embedding table and the matching slice of
counts^T, computes its partial pooled on the tensor engine (bf16 x bf16
-> fp32 PSUM), and a bf16 ReduceScatter sums the partials and hands core
i batch rows [128*i, 128*(i+1)).  The MLP tail runs per-core on its 128
batch rows: pooled^T lands via XBAR DMA-transpose straight out of the
collective buffer, fc1 computes h^T = relu(W1^T pooled^T + b1) so fc2
(out = h @ W2 + b2) needs no transposes at all; bf16 inputs, fp32 PSUM.

Schedule notes: dummy matmuls on memset tiles ramp the PE pstate during
the initial DMA fill (the real accumulation opens with start=True, so
the junk is discarded); count/embedding DMAs interleave across the two
HWDGE queues (sync + scalar; counts two 128-row chunks per instruction)
while gpsimd carries only the small transfers, keeping every issue path
ahead of the PE's ~1.0 us/chunk consume rate.  The ReduceScatter
triggers as soon as the accumulator drains land; its start is pinned by
NRT's fixed first-collective barrier (~70 us), which the matmul phase
hides.
"""

import sys

import numpy as np

for _p in ("/opt/trn_rl_repo",):
    if _p not in sys.path:
        sys.path.insert(0, _p)

V, E, H, O = 50000, 300, 512, 2
S, B = 512, 1024
NCORES = 8
VSH = 6272          # padded vocab rows per core (49 * 128)
VP = NCORES * VSH   # 50176 padded vocab rows total
KC = VSH // 128     # 49 contraction chunks per core
BG = B // 128       # 8 batch groups of 128
BS = B // NCORES    # 128 batch rows per core after reduce-scatter
NWARM = 8           # dummy matmuls to start the PE pstate ramp
OV = 256 - 172      # zeroed overlap rows in the third fc1 weight chunk


def _build_nc(repeat=None, cnt_fp8=True):
    import os
    from contextlib import ExitStack

    if repeat is None:
        repeat = int(os.environ.get("KERNEL_REPEAT", "1"))

    import concourse.tile as tile
    from concourse import bacc, bass, mybir

    bf16, f32 = mybir.dt.bfloat16, mybir.dt.float32
    cdt = mybir.dt.float8e4 if cnt_fp8 else bf16

    nc = bacc.Bacc(None, target_bir_lowering=False, num_devices=NCORES)
    cnt_d = nc.declare_dram_parameter("cnt", [VSH, B], cdt, isOutput=False)
    emb_d = nc.declare_dram_parameter("emb", [VSH, E], bf16, isOutput=False)
    il_d = nc.declare_dram_parameter("ivl", [128, BG], f32, isOutput=False)
    # rows 0:384 = three 128-row lhsT chunks of W1 (the third covers
    # pooled^T rows 172:300 with the 84 overlap rows zeroed), row 384 = b1
    w1b_d = nc.declare_dram_parameter("w1b", [385, H], bf16, isOutput=False)
    w2b_d = nc.declare_dram_parameter("w2b", [H + 1, O], bf16, isOutput=False)
    out_d = nc.declare_dram_parameter("out", [BS, O], f32, isOutput=True)

    with tile.TileContext(nc) as tc, ExitStack() as ctx:
        sb = ctx.enter_context(tc.tile_pool(name="sb", bufs=1))
        dram = ctx.enter_context(tc.tile_pool(name="dram", bufs=1, space="DRAM"))

        # tiny warm-up collective, triggered as early as possible: every
        # core reaches NRT's first-collective rendezvous within ~10 us of
        # its NEFF start, so cross-core launch skew and the CC stream
        # setup are absorbed while the matmul phase runs (the real
        # ReduceScatter then starts trigger-bound, not barrier-bound).
        # The input is an uninitialized scratch tile (nobody reads the
        # result), so the trigger carries no data dependency at all.
        warm_in = dram.tile([8, 64], bf16)
        warm_out = dram.tile([1, 64], bf16)
        nc.gpsimd.collective_compute(
            "ReduceScatter",
            mybir.AluOpType.add,
            replica_groups=[list(range(NCORES))],
            ins=[warm_in.opt()],
            outs=[warm_out.opt()],
        )

        # counts (two 128-row chunks per instruction) and embeddings,
        # interleaved in chunk order across the two HWDGE queues
        cnt_t, emb_t = [], []
        for j in range((KC + 1) // 2):
            r1 = min((j + 1) * 256, VSH)
            t2 = (r1 - j * 256) // 128
            ct = sb.tile([128, t2 * B], cdt, tag=f"cnt{j}", name=f"cnt{j}")
            eng_c = nc.sync if j % 2 == 0 else nc.scalar
            eng_e = nc.scalar if j % 2 == 0 else nc.sync
            eng_c.dma_start(
                out=ct[:].rearrange("p (t c) -> p t c", t=t2),
                in_=cnt_d[j * 256:r1, :].rearrange("(t p) c -> p t c", t=t2),
            )
            cnt_t.append(ct)
            for k in range(2 * j, 2 * j + t2):
                et = sb.tile([128, E], bf16, tag=f"emb{k}", name=f"emb{k}")
                eng_e.dma_start(out=et[:],
                                in_=emb_d[k * 128:(k + 1) * 128, :])
                emb_t.append(et)

        w1_t = []
        for c in range(3):
            t = sb.tile([128, H], bf16, tag=f"w1_{c}", name=f"w1_{c}")
            nc.gpsimd.dma_start(out=t[:], in_=w1b_d[c * 128:(c + 1) * 128, :])
            w1_t.append(t)
        b1_t = sb.tile([1, H], bf16, tag="b1")
        nc.gpsimd.dma_start(out=b1_t[:], in_=w1b_d[384:385, :])
        w2_t = []
        for c in range(4):
            t = sb.tile([128, O], bf16, tag=f"w2_{c}", name=f"w2_{c}")
            nc.gpsimd.dma_start(out=t[:], in_=w2b_d[c * 128:(c + 1) * 128, :])
            w2_t.append(t)
        b2_t = sb.tile([1, O], bf16, tag="b2")
        nc.gpsimd.dma_start(out=b2_t[:], in_=w2b_d[H:H + 1, :])
        ivl = sb.tile([128, BG], f32, tag="ivl")
        nc.gpsimd.dma_start(out=ivl[:], in_=il_d[:])

        # PE pstate warm-up on memset tiles (no DMA dependency); the real
        # accumulation below opens with start=True, discarding this junk
        wa = sb.tile([128, 128], bf16, tag="wa")
        nc.vector.memset(wa[:], 0.0)
        wb = sb.tile([128, E], bf16, tag="wb")
        nc.vector.memset(wb[:], 0.0)
        ones1 = sb.tile([1, 128], bf16, tag="ones1")
        nc.vector.memset(ones1[:], 1.0)

        pooled_all = sb.tile([128, BG * E], bf16, tag="pooled_all")
        with tc.tile_pool(name="psA", bufs=1, space="PSUM") as psA:
            acc = [
                psA.tile([128, 512], f32, tag=f"acc{g}", name=f"acc{g}")
                for g in range(BG)
            ]
            for w in range(NWARM):
                nc.tensor.matmul(out=acc[0][:, 0:E], lhsT=wa[:], rhs=wb[:],
                                 start=True, stop=True)
            # group order: bounce piece A's groups (4-7) stop and drain
            # first, piece B's (0-2) next, and g3 last so the trigger's
            # final wait covers only the tiny last piece
            GORD = (4, 5, 6, 7, 0, 1, 2, 3)
            for rep in range(repeat):
                for k in range(KC):
                    ct = cnt_t[k // 2]
                    t = k % 2
                    for g in GORD:
                        nc.tensor.matmul(
                            out=acc[g][:, 0:E],
                            lhsT=ct[:, t * B + g * 128:t * B + (g + 1) * 128],
                            rhs=emb_t[k][:],
                            start=(k == 0),
                            stop=(k == KC - 1),
                        )
            # drain the accumulators, folding in the 1/len scale (vector +
            # scalar in parallel; pipelines behind the last matmuls)
            for g in GORD:
                dst = pooled_all[:, g * E:(g + 1) * E]
                if g % 2 == 0:
                    nc.vector.tensor_scalar(
                        out=dst, in0=acc[g][:, 0:E],
                        scalar1=ivl[:, g:g + 1], scalar2=None,
                        op0=mybir.AluOpType.mult,
                    )
                else:
                    nc.scalar.activation(
                        out=dst, in_=acc[g][:, 0:E],
                        func=mybir.ActivationFunctionType.Copy,
                        scale=ivl[:, g:g + 1],
                    )

        # cross-core sum + scatter: core i keeps batch rows [128i, 128i+128).
        # Bounce in three pieces across three queues, staged behind the
        # drains, so the collective trigger's last completion wait is a
        # single-group DMA.
        part_d = dram.tile([B, E], bf16)
        rs_d = dram.tile([BS, E], bf16)
        nc.gpsimd.dma_start(
            out=part_d[4 * 128:, :].rearrange("(g p) e -> p g e", g=4),
            in_=pooled_all[:, 4 * E:].rearrange("p (g e) -> p g e", g=4),
        )
        nc.sync.dma_start(
            out=part_d[0:3 * 128, :].rearrange("(g p) e -> p g e", g=3),
            in_=pooled_all[:, 0:3 * E].rearrange("p (g e) -> p g e", g=3),
        )
        nc.scalar.dma_start(
            out=part_d[3 * 128:4 * 128, :],
            in_=pooled_all[:, 3 * E:4 * E],
        )
        nc.gpsimd.collective_compute(
            "ReduceScatter",
            mybir.AluOpType.add,
            replica_groups=[list(range(NCORES))],
            ins=[part_d.opt()],
            outs=[rs_d.opt()],
        )

        with tc.tile_pool(name="ps", bufs=1, space="PSUM") as ps:
            # pooled^T as three 128-wide XBAR DMA-transposes straight from
            # rs_d; the third chunk (cols 172:300) overlaps the second, and
            # the matching fc1 weight rows are zeroed host-side
            lhs = []
            for c, c0 in enumerate([0, 128, E - 128]):
                lt = sb.tile([128, 128], bf16, tag=f"lhs{c}", name=f"lhs{c}")
                eng = nc.scalar if c == 1 else nc.sync
                eng.dma_start_transpose(lt[:], rs_d[:, c0:c0 + 128])
                lhs.append(lt)

            # fc1 flipped: hT[hc] = W1[:, hc]^T @ pooled^T, bias via ones row
            hT_ps = [
                ps.tile([128, 128], f32, tag=f"hT{hc}", name=f"hT{hc}")
                for hc in range(4)
            ]
            for hc in range(4):
                for c in range(3):
                    nc.tensor.matmul(
                        out=hT_ps[hc][:],
                        lhsT=w1_t[c][:, hc * 128:(hc + 1) * 128],
                        rhs=lhs[c][:],
                        start=(c == 0), stop=False,
                    )
                nc.tensor.matmul(
                    out=hT_ps[hc][:],
                    lhsT=b1_t[:, hc * 128:(hc + 1) * 128],
                    rhs=ones1[:],
                    start=False, stop=True,
                )
            hT = sb.tile([128, 4 * 128], bf16, tag="hT")
            for hc in range(4):
                dst = hT[:, hc * 128:(hc + 1) * 128]
                if hc % 2 == 0:
                    nc.scalar.activation(
                        out=dst, in_=hT_ps[hc][:],
                        func=mybir.ActivationFunctionType.Relu)
                else:
                    nc.vector.tensor_scalar(
                        out=dst, in0=hT_ps[hc][:], scalar1=0.0, scalar2=None,
                        op0=mybir.AluOpType.max)

            # fc2: out = h @ W2 + b2 (hT is already the needed lhsT)
            op_ = ps.tile([128, O], f32, tag="op", space="PSUM")
            for c in range(4):
                nc.tensor.matmul(
                    out=op_[:], lhsT=hT[:, c * 128:(c + 1) * 128],
                    rhs=w2_t[c][:], start=(c == 0), stop=False)
            nc.tensor.matmul(out=op_[:], lhsT=ones1[:], rhs=b2_t[:],
                             start=False, stop=True)
            out_sb = sb.tile([BS, O], f32, tag="osb")
            nc.vector.tensor_copy(out=out_sb[:], in_=op_[:])
            # scalar HWDGE queue: still warm from the XBAR transpose a few
            # us earlier, so the completion semaphore lands in ~2 us instead
            # of the ~6 us an idle queue takes
            nc.scalar.dma_start(out=out_d[:], in_=out_sb[:])

    nc.finalize()
    return nc


def _prep_in_maps(text, lengths, emb_table, W1, b1, W2, b2):
    import ml_dtypes

    bf16 = ml_dtypes.bfloat16
    text = np.asarray(text, dtype=np.int64)         # [S, B]
    lengths = np.asarray(lengths, dtype=np.int64)   # [B]

    # counts^T [VP, B]: row v = per-batch count of token v among the
    # first len[b] positions (vocab-major for sharding); the 1/len scale
    # is applied on-device at accumulator-drain time
    mask = np.arange(S)[:, None] < lengths[None, :]
    flat = (text * B + np.arange(B)[None, :])[mask]
    cntT = np.bincount(flat, minlength=VP * B).reshape(VP, B)
    cnt_fp8 = cntT.max() <= 15  # integers <= 15 are exact in fp8e4m3
    cdt = ml_dtypes.float8_e4m3fn if cnt_fp8 else bf16
    cntT16 = cntT.astype(cdt)
    inv_len = (1.0 / lengths.astype(np.float32)).astype(np.float32)
    ivl = np.ascontiguousarray(inv_len.reshape(BG, 128).T)  # [128, BG]

    embp = np.zeros((VP, E), np.float32)
    embp[:V] = np.asarray(emb_table, np.float32)
    emb16 = embp.astype(bf16)

    # fc1 lhsT chunks: [0:128), [128:256), [172:300) with the 84 rows that
    # overlap chunk 1 zeroed, then the b1 row
    W1f = np.asarray(W1, np.float32)
    c2 = np.zeros((128, H), np.float32)
    c2[OV:] = W1f[256:E]
    w1b = np.vstack([W1f[0:128], W1f[128:256], c2,
                     np.asarray(b1, np.float32)[None, :]]).astype(bf16)
    w2b = np.vstack([np.asarray(W2, np.float32),
                     np.asarray(b2, np.float32)[None, :]]).astype(bf16)

    in_maps = []
    for i in range(NCORES):
        in_maps.append({
            "cnt": np.ascontiguousarray(cntT16[i * VSH:(i + 1) * VSH]),
            "emb": np.ascontiguousarray(emb16[i * VSH:(i + 1) * VSH]),
            "ivl": ivl,
            "w1b": w1b,
            "w2b": w2b,
        })
    return in_maps, cnt_fp8


def _run(inputs, trace=False):
    from concourse.bass_utils import run_bass_kernel_spmd

    in_maps, cnt_fp8 = _prep_in_maps(**inputs)
    nc = _build_nc(cnt_fp8=cnt_fp8)
    res = run_bass_kernel_spmd(nc, in_maps, list(range(NCORES)), trace=trace)
    out = np.concatenate([res.results[i]["out"] for i in range(NCORES)], axis=0)
    return out.astype(np.float32), res


def kernel(**inputs):
    out, _ = _run(inputs, trace=False)
    return out


# revision 50
# speedup vs baseline: 1.6244x; 1.1410x over previous
"""BOW classifier kernel for 8 Trainium2 NeuronCores.

Vocab-sharded counts-matmul formulation.  The masked mean-pool
  pooled[b] = (1/len[b]) * sum_{s<len[b]} emb[text[s,b]]
is a sparse matmul  pooled = counts @ emb  with counts[b,v] the number of
times token v appears in the first len[b] positions of column b.  Each
core owns a 6272-row slice of the (padded, bf16) embedding table and the
matching slice of integer counts^T in fp8e4m3 (exact for counts <= 15,
with a bf16 fallback), computes its partial pooled on the tensor engine
(fp8 x bf16 -> fp32 PSUM), folds the 1/len scale into the PSUM-drain
step, and a bf16 ReduceScatter sums the partials and hands core i batch
rows [128*i, 128*(i+1)).  The MLP tail runs per-core on its 128 batch
rows: pooled^T lands via XBAR DMA-transpose straight out of the
collective buffer, fc1 computes h^T = relu(W1^T pooled^T + b1) so fc2
(out = h @ W2 + b2) needs no transposes at all; bf16 inputs, fp32 PSUM.

Schedule notes: a tiny warm-up ReduceScatter on an uninitialized scratch
tile fires in the first ~8 us, so NRT's first-collective barrier (which
absorbs cross-core launch skew) and the CC stream setup run concurrently
with the matmul phase, and the real collective starts trigger-bound with
~1-2 us prep instead of ~11.5 us cold.  Dummy matmuls on memset tiles
ramp the PE pstate during the initial DMA fill (the real accumulation
opens with start=True, so the junk is discarded); count DMAs move two
128-row chunks per instruction, interleaved with embedding DMAs across
the two HWDGE queues (sync + scalar) while gpsimd carries only small
transfers, keeping every issue path ahead of the PE's ~1.0 us/chunk
consume rate.  The accumulator drains run on vector+scalar in bounce-
piece order, the PSUM->DRAM bounce ships in three pieces across three
queues so the trigger's last completion wait is a single-group DMA, and
the output DMA uses the still-warm scalar queue.
"""

import sys

import numpy as np

for _p in ("/opt/trn_rl_repo",):
    if _p not in sys.path:
        sys.path.insert(0, _p)

V, E, H, O = 50000, 300, 512, 2
S, B = 512, 1024
NCORES = 8
VSH = 6272          # padded vocab rows per core (49 * 128)
VP = NCORES * VSH   # 50176 padded vocab rows total
KC = VSH // 128     # 49 contraction chunks per core
BG = B // 128       # 8 batch groups of 128
BS = B // NCORES    # 128 batch rows per core after reduce-scatter
NWARM = 8           # dummy matmuls to start the PE pstate ramp
OV = 256 - 172      # zeroed overlap rows in the third fc1 weight chunk


def _build_nc(repeat=None, cnt_fp8=True):
    import os
    from contextlib import ExitStack

    if repeat is None:
        repeat = int(os.environ.get("KERNEL_REPEAT", "1"))

    import concourse.tile as tile
    from concourse import bacc, bass, mybir

    bf16, f32 = mybir.dt.bfloat16, mybir.dt.float32
    cdt = mybir.dt.float8e4 if cnt_fp8 else bf16

    nc = bacc.Bacc(None, target_bir_lowering=False, num_devices=NCORES)
    cnt_d = nc.declare_dram_parameter("cnt", [VSH, B], cdt, isOutput=False)
    emb_d = nc.declare_dram_parameter("emb", [VSH, E], bf16, isOutput=False)
    il_d = nc.declare_dram_parameter("ivl", [128, BG], f32, isOutput=False)
    # rows 0:384 = three 128-row lhsT chunks of W1 (the third covers
    # pooled^T rows 172:300 with the 84 overlap rows zeroed), row 384 = b1
    w1b_d = nc.declare_dram_parameter("w1b", [385, H], bf16, isOutput=False)
    w2b_d = nc.declare_dram_parameter("w2b", [H + 1, O], bf16, isOutput=False)
    out_d = nc.declare_dram_parameter("out", [BS, O], f32, isOutput=True)

    with tile.TileContext(nc) as tc, ExitStack() as ctx:
        sb = ctx.enter_context(tc.tile_pool(name="sb", bufs=1))
        dram = ctx.enter_context(tc.tile_pool(name="dram", bufs=1, space="DRAM"))

        # tiny warm-up collective, triggered as early as possible: every
        # core reaches NRT's first-collective rendezvous within ~10 us of
        # its NEFF start, so cross-core launch skew and the CC stream
        # setup are absorbed while the matmul phase runs (the real
        # ReduceScatter then starts trigger-bound, not barrier-bound).
        # The input is an uninitialized scratch tile (nobody reads the
        # result), so the trigger carries no data dependency at all.
        warm_in = dram.tile([8, 64], bf16)
        warm_out = dram.tile([1, 64], bf16)
        nc.gpsimd.collective_compute(
            "ReduceScatter",
            mybir.AluOpType.add,
            replica_groups=[list(range(NCORES))],
            ins=[warm_in.opt()],
            outs=[warm_out.opt()],
        )

        # counts (two 128-row chunks per instruction) and embeddings,
        # interleaved in chunk order across the two HWDGE queues
        cnt_t, emb_t = [], []
        for j in range((KC + 1) // 2):
            r1 = min((j + 1) * 256, VSH)
            t2 = (r1 - j * 256) // 128
            ct = sb.tile([128, t2 * B], cdt, tag=f"cnt{j}", name=f"cnt{j}")
            eng_c = nc.sync if j % 2 == 0 else nc.scalar
            eng_e = nc.scalar if j % 2 == 0 else nc.sync
            eng_c.dma_start(
                out=ct[:].rearrange("p (t c) -> p t c", t=t2),
                in_=cnt_d[j * 256:r1, :].rearrange("(t p) c -> p t c", t=t2),
            )
            cnt_t.append(ct)
            for k in range(2 * j, 2 * j + t2):
                et = sb.tile([128, E], bf16, tag=f"emb{k}", name=f"emb{k}")
                eng_e.dma_start(out=et[:],
                                in_=emb_d[k * 128:(k + 1) * 128, :])
                emb_t.append(et)

        w1_t = []
        for c in range(3):
            t = sb.tile([128, H], bf16, tag=f"w1_{c}", name=f"w1_{c}")
            nc.gpsimd.dma_start(out=t[:], in_=w1b_d[c * 128:(c + 1) * 128, :])
            w1_t.append(t)
        b1_t = sb.tile([1, H], bf16, tag="b1")
        nc.gpsimd.dma_start(out=b1_t[:], in_=w1b_d[384:385, :])
        w2_t = []
        for c in range(4):
            t = sb.tile([128, O], bf16, tag=f"w2_{c}", name=f"w2_{c}")
            nc.gpsimd.dma_start(out=t[:], in_=w2b_d[c * 128:(c + 1) * 128, :])
            w2_t.append(t)
        b2_t = sb.tile([1, O], bf16, tag="b2")
        nc.gpsimd.dma_start(out=b2_t[:], in_=w2b_d[H:H + 1, :])
        ivl = sb.tile([128, BG], f32, tag="ivl")
        nc.gpsimd.dma_start(out=ivl[:], in_=il_d[:])

        # PE pstate warm-up on memset tiles (no DMA dependency); the real
        # accumulation below opens with start=True, discarding this junk
        wa = sb.tile([128, 128], bf16, tag="wa")
        nc.vector.memset(wa[:], 0.0)
        wb = sb.tile([128, E], bf16, tag="wb")
        nc.vector.memset(wb[:], 0.0)
        ones1 = sb.tile([1, 128], bf16, tag="ones1")
        nc.vector.memset(ones1[:], 1.0)

        pooled_all = sb.tile([128, BG * E], bf16, tag="pooled_all")
        with tc.tile_pool(name="psA", bufs=1, space="PSUM") as psA:
            acc = [
                psA.tile([128, 512], f32, tag=f"acc{g}", name=f"acc{g}")
                for g in range(BG)
            ]
            for w in range(NWARM):
                nc.tensor.matmul(out=acc[0][:, 0:E], lhsT=wa[:], rhs=wb[:],
                                 start=True, stop=True)
            # group order: bounce piece A's groups (4-7) stop and drain
            # first, piece B's (0-2) next, and g3 last so the trigger's
            # final wait covers only the tiny last piece
            GORD = (4, 5, 6, 7, 0, 1, 2, 3)
            for rep in range(repeat):
                for k in range(KC):
                    ct = cnt_t[k // 2]
                    t = k % 2
                    for g in GORD:
                        nc.tensor.matmul(
                            out=acc[g][:, 0:E],
                            lhsT=ct[:, t * B + g * 128:t * B + (g + 1) * 128],
                            rhs=emb_t[k][:],
                            start=(k == 0),
                            stop=(k == KC - 1),
                        )
            # drain the accumulators, folding in the 1/len scale (vector +
            # scalar in parallel; pipelines behind the last matmuls)
            for g in GORD:
                dst = pooled_all[:, g * E:(g + 1) * E]
                if g % 2 == 0:
                    nc.vector.tensor_scalar(
                        out=dst, in0=acc[g][:, 0:E],
                        scalar1=ivl[:, g:g + 1], scalar2=None,
                        op0=mybir.AluOpType.mult,
                    )
                else:
                    nc.scalar.activation(
                        out=dst, in_=acc[g][:, 0:E],
                        func=mybir.ActivationFunctionType.Copy,
                        scale=ivl[:, g:g + 1],
                    )

        # cross-core sum + scatter: core i keeps batch rows [128i, 128i+128).
        # Bounce in three pieces across three queues, staged behind the
        # drains, so the collective trigger's last completion wait is a
        # single-group DMA.
        part_d = dram.tile([B, E], bf16)
        rs_d = dram.tile([BS, E], bf16)
        nc.gpsimd.dma_start(
            out=part_d[4 * 128:, :].rearrange("(g p) e -> p g e", g=4),
            in_=pooled_all[:, 4 * E:].rearrange("p (g e) -> p g e", g=4),
        )
        nc.sync.dma_start(
            out=part_d[0:3 * 128, :].rearrange("(g p) e -> p g e", g=3),
            in_=pooled_all[:, 0:3 * E].rearrange("p (g e) -> p g e", g=3),
        )
        nc.scalar.dma_start(
            out=part_d[3 * 128:4 * 128, :],
            in_=pooled_all[:, 3 * E:4 * E],
        )
        nc.gpsimd.collective_compute(
            "ReduceScatter",
            mybir.AluOpType.add,
            replica_groups=[list(range(NCORES))],
            ins=[part_d.opt()],
            outs=[rs_d.opt()],
        )

        with tc.tile_pool(name="ps", bufs=1, space="PSUM") as ps:
            # pooled^T as three 128-wide XBAR DMA-transposes straight from
            # rs_d; the third chunk (cols 172:300) overlaps the second, and
            # the matching fc1 weight rows are zeroed host-side
            lhs = []
            for c, c0 in enumerate([0, 128, E - 128]):
                lt = sb.tile([128, 128], bf16, tag=f"lhs{c}", name=f"lhs{c}")
                eng = nc.scalar if c == 1 else nc.sync
                eng.dma_start_transpose(lt[:], rs_d[:, c0:c0 + 128])
                lhs.append(lt)

            # fc1 flipped: hT[hc] = W1[:, hc]^T @ pooled^T, bias via ones row
            hT_ps = [
                ps.tile([128, 128], f32, tag=f"hT{hc}", name=f"hT{hc}")
                for hc in range(4)
            ]
            for hc in range(4):
                for c in range(3):
                    nc.tensor.matmul(
                        out=hT_ps[hc][:],
                        lhsT=w1_t[c][:, hc * 128:(hc + 1) * 128],
                        rhs=lhs[c][:],
                        start=(c == 0), stop=False,
                    )
                nc.tensor.matmul(
                    out=hT_ps[hc][:],
                    lhsT=b1_t[:, hc * 128:(hc + 1) * 128],
                    rhs=ones1[:],
                    start=False, stop=True,
                )
            hT = sb.tile([128, 4 * 128], bf16, tag="hT")
            for hc in range(4):
                dst = hT[:, hc * 128:(hc + 1) * 128]
                if hc % 2 == 0:
                    nc.scalar.activation(
                        out=dst, in_=hT_ps[hc][:],
                        func=mybir.ActivationFunctionType.Relu)
                else:
                    nc.vector.tensor_scalar(
                        out=dst, in0=hT_ps[hc][:], scalar1=0.0, scalar2=None,
                        op0=mybir.AluOpType.max)

            # fc2: out = h @ W2 + b2 (hT is already the needed lhsT)
            op_ = ps.tile([128, O], f32, tag="op", space="PSUM")
            for c in range(4):
                nc.tensor.matmul(
                    out=op_[:], lhsT=hT[:, c * 128:(c + 1) * 128],
                    rhs=w2_t[c][:], start=(c == 0), stop=False)
            nc.tensor.matmul(out=op_[:], lhsT=ones1[:], rhs=b2_t[:],
                             start=False, stop=True)
            out_sb = sb.tile([BS, O], f32, tag="osb")
            nc.vector.tensor_copy(out=out_sb[:], in_=op_[:])
            # scalar HWDGE queue: still warm from the XBAR transpose a few
            # us earlier, so the completion semaphore lands in ~2 us instead
            # of the ~6 us an idle queue takes
            nc.scalar.dma_start(out=out_d[:], in_=out_sb[:])

    nc.finalize()
    return nc


def _prep_in_maps(text, lengths, emb_table, W1, b1, W2, b2):
    import ml_dtypes

    bf16 = ml_dtypes.bfloat16
    text = np.asarray(text, dtype=np.int64)         # [S, B]
    lengths = np.asarray(lengths, dtype=np.int64)   # [B]

    # counts^T [VP, B]: row v = per-batch count of token v among the
    # first len[b] positions (vocab-major for sharding); the 1/len scale
    # is applied on-device at accumulator-drain time
    mask = np.arange(S)[:, None] < lengths[None, :]
    flat = (text * B + np.arange(B)[None, :])[mask]
    cntT = np.bincount(flat, minlength=VP * B).reshape(VP, B)
    cnt_fp8 = cntT.max() <= 15  # integers <= 15 are exact in fp8e4m3
    cdt = ml_dtypes.float8_e4m3fn if cnt_fp8 else bf16
    cntT16 = cntT.astype(cdt)
    inv_len = (1.0 / lengths.astype(np.float32)).astype(np.float32)
    ivl = np.ascontiguousarray(inv_len.reshape(BG, 128).T)  # [128, BG]

    embp = np.zeros((VP, E), np.float32)
    embp[:V] = np.asarray(emb_table, np.float32)
    emb16 = embp.astype(bf16)

    # fc1 lhsT chunks: [0:128), [128:256), [172:300) with the 84 rows that
    # overlap chunk 1 zeroed, then the b1 row
    W1f = np.asarray(W1, np.float32)
    c2 = np.zeros((128, H), np.float32)
    c2[OV:] = W1f[256:E]
    w1b = np.vstack([W1f[0:128], W1f[128:256], c2,
                     np.asarray(b1, np.float32)[None, :]]).astype(bf16)
    w2b = np.vstack([np.asarray(W2, np.float32),
                     np.asarray(b2, np.float32)[None, :]]).astype(bf16)

    in_maps = []
    for i in range(NCORES):
        in_maps.append({
            "cnt": np.ascontiguousarray(cntT16[i * VSH:(i + 1) * VSH]),
            "emb": np.ascontiguousarray(emb16[i * VSH:(i + 1) * VSH]),
            "ivl": ivl,
            "w1b": w1b,
            "w2b": w2b,
        })
    return in_maps, cnt_fp8


def _run(inputs, trace=False):
    from concourse.bass_utils import run_bass_kernel_spmd

    in_maps, cnt_fp8 = _prep_in_maps(**inputs)
    nc = _build_nc(cnt_fp8=cnt_fp8)
    res = run_bass_kernel_spmd(nc, in_maps, list(range(NCORES)), trace=trace)
    out = np.concatenate([res.results[i]["out"] for i in range(NCORES)], axis=0)
    return out.astype(np.float32), res


def kernel(**inputs):
    out, _ = _run(inputs, trace=False)
    return out
